# revision 15
# baseline (speedup 1.0000x reference)
"""AspectAttention Trainium2 kernel (8 NeuronCores, batch-parallel, fp8).

out = tok * (1 + softmax_S(tanh(cat(tok, mean_A(asp)) @ W + b) @ v))

Sharding: data-parallel over batch B=16 -> 2 batches per core. Softmax is
per-(batch) row over S, so no cross-core communication is needed.

Per-core math (concat split): E^T = tanh(W1^T @ X^T + biasT), where
biasT = (mean_A(asp) @ W2 + b) is per-batch and precomputed on the host
(a 64KB constant, 0.03% of the FLOPs). scores = v^T @ E^T;
weights = softmax(scores); out = X * (1 + weights).

Layout strategy (v2): all transposes and dtype casts are done on the host.
The device receives
  - tokT  [NB, 128, KP, 2, 512] fp8: X^T pre-transposed and pre-cast, laid
    out as DoubleRow k-pair rhs slabs (one contiguous 512KB chunk per
    512-token block),
  - w8    [128, KP, 2, H] fp8: 64*W1 as DoubleRow lhsT chunks (1MB),
  - tokh  [NT, 128, H] fp16: X natural-layout for the output scaling
    (fp16 is lossless enough: ~5e-4 elementwise vs the 2e-2 gate),
  - biasT [H, BPC] f32, v8 [128, MC, 16] fp8.
This removes all PE transpose traffic and all DVE cast traffic from the
device and cuts HBM reads from 21MB to 13.6MB per core. The PE stream is
then almost pure DoubleRow matmuls at ~216ns per [256 x 128 x 512] MM.

Schedule notes:
- v-dots run as fp8 DoubleRow MMs over pair-interleaved e tiles (tanh
  writes fp8 directly into the two Ko slabs), deferred one block so the
  PSUM->tanh->vdot chain hides inside the next block's MM stream.
- scores are staged as [4, 512] rows per batch and transposed in four
  [4,128] PE transposes at batch end (N=4 each, ~free).
- block 0 runs kp-outer over m0-3 so MMs start as soon as the first w8
  chunk + tokT block land (~3us); ~18 warmup MMs keep HAM busy before.
- batch-0 scale+stores spread over blocks 5-7; batch-1 is the tail
  (8.4MB of stores, write-BW-bound).
"""

from contextlib import ExitStack

import numpy as np
import ml_dtypes

import concourse.bass as bass
import concourse.mybir as mybir
import concourse.tile as tile
from concourse import bacc, bass_isa
from concourse.bass_utils import run_bass_kernel_spmd
from concourse.masks import make_identity

B, S, H, A = 16, 2048, 1024, 8
NCORES = 8
BPC = B // NCORES          # batches per core = 2
T = BPC * S                # tokens per core = 4096
NT = T // 128              # 32 token-128 tiles per core
NB = T // 512              # 8 token-512 blocks per core
KP = 4                     # double-row k-pairs (contraction 1024 = 4*256)
MC = H // 128              # 8 output-dim chunks
WSCALE = 64.0              # W1 fp8 pre-scale

F32 = mybir.dt.float32
F16 = mybir.dt.float16
F8 = mybir.dt.float8e4
ALU = mybir.AluOpType
AF = mybir.ActivationFunctionType
AX = mybir.AxisListType
DR = mybir.MatmulPerfMode.DoubleRow

F8NP = ml_dtypes.float8_e4m3


def _emit(ctx: ExitStack, tc: "tile.TileContext", out, tokT, w8, tokh,
          biasT_in, v8_in):
    nc = tc.nc

    const = ctx.enter_context(tc.tile_pool(name="const", bufs=1))
    thp = ctx.enter_context(tc.tile_pool(name="thp", bufs=28))
    ep = ctx.enter_context(tc.tile_pool(name="ep", bufs=10))
    stp = ctx.enter_context(tc.tile_pool(name="stp", bufs=6))
    smp = ctx.enter_context(tc.tile_pool(name="smp", bufs=1))

    # PSUM: 8 banks, bank-granular: mm 4 + vd 2 + tp 2
    mm_ps = ctx.enter_context(tc.tile_pool(name="mm_ps", bufs=4, space="PSUM"))
    vd_ps = ctx.enter_context(tc.tile_pool(name="vd_ps", bufs=2, space="PSUM"))
    tp_ps = ctx.enter_context(tc.tile_pool(name="tp_ps", bufs=2, space="PSUM"))

    # ---- HAM warmup: dummy matmuls keep the PE busy from t=0 while the
    # first DMAs land. The result is copied into a scratch tile no one
    # reads for real.
    warm = const.tile([128, 512], F8)
    nc.vector.memset(warm[:], 0.0)
    scratch = const.tile([128, 1], F32)
    wps = None
    for r in range(18):
        wps = mm_ps.tile([128, 256], F32, tag="mm", name=f"warm{r % 4}")
        nc.tensor.matmul(wps[:], warm[:, 0:128], warm[:, 0:256],
                         start=True, stop=True, skip_group_check=True)
    nc.vector.tensor_copy(out=scratch[:], in_=wps[:, 0:1])

    # ---- constants / small inputs ------------------------------------
    ident = const.tile([128, 128], F32)
    make_identity(nc, ident[:])

    biasT = const.tile([128, MC, BPC], F32)      # bias[b, m*128+p] at [p,m,b]
    nc.scalar.dma_start(biasT[:], biasT_in.rearrange("(m p) b -> p m b", p=128))
    v8 = const.tile([128, MC, 16], F8)           # v[m*128+p] at [p, m, 0]
    nc.scalar.dma_start(v8[:], v8_in)

    # ---- bulk input tiles --------------------------------------------
    w8_sb = const.tile([128, KP, 2, H], F8)      # 64*W1 DR lhsT chunks
    tokT_sb = const.tile([128, NB, KP, 2, 512], F8)  # X^T DR rhs slabs

    nc.sync.dma_start(tokT_sb[:, 0], tokT[0])
    nc.gpsimd.dma_start(w8_sb[:, 0], w8[:, 0])
    nc.sync.dma_start(w8_sb[:, 1], w8[:, 1])
    nc.gpsimd.dma_start(w8_sb[:, 2], w8[:, 2])
    nc.sync.dma_start(tokT_sb[:, 1], tokT[1])
    nc.gpsimd.dma_start(w8_sb[:, 3], w8[:, 3])
    for t in range(2, NB):
        (nc.sync if t % 2 == 0 else nc.gpsimd).dma_start(
            tokT_sb[:, t], tokT[t])

    out_t = out.rearrange("(n p) h -> n p h", p=128)

    s_sb = const.tile([128, NT], F32)   # per-token-tile (1 + weight) scales
    tokh_sb = {}                        # n -> [128, H] f16 tile
    e8 = {}                             # (t, q) -> [128, 2, 512] f8 tile
    vd_row = {}                         # t -> [1, 512] psum score row
    sc4 = {}                            # bb -> [128, 512] score rows (SBUF)
    sT = {}                             # bb -> [128, 16] transposed scores
    for bb in range(BPC):
        sc4[bb] = smp.tile([128, 512], F32, tag=f"sc4_{bb}", name=f"sc4_{bb}")
        nc.vector.memset(sc4[bb][:], 0.0)
        sT[bb] = smp.tile([128, 16], F32, tag=f"sT{bb}", name=f"sT{bb}")

    def load_tokh(n, eng):
        t_ = thp.tile([128, H], F16, tag="th", name=f"th{n}")
        tokh_sb[n] = t_
        eng.dma_start(t_[:], tokh[n])

    def emit_vdot(t, q):
        if q == 0:
            vd_row[t] = vd_ps.tile([1, 512], F32, tag="vd", name=f"vd{t}")
        nc.tensor.matmul(
            vd_row[t][:], v8[:, 2 * q : 2 * q + 2, 0:1], e8[(t, q)][:],
            start=(q == 0), stop=(q == 3), perf_mode=DR,
            skip_group_check=True)
        if q == 3:
            # copy the finished row to partition 32*(t%4) of the batch's
            # score tile (engine writes must start 32-aligned)
            bb, r = divmod(t, 4)
            nc.scalar.copy(sc4[bb][32 * r : 32 * r + 1, :], vd_row[t][:])

    def emit_score_tp(bb, jj):
        tp = tp_ps.tile([128, 128], F32, tag="tp", name=f"tp{bb}_{jj}")
        nc.tensor.transpose(
            tp[:], sc4[bb][:, jj * 128 : (jj + 1) * 128], ident[:])
        nc.vector.tensor_copy(
            out=sT[bb][:].rearrange("p (b j) -> p b j", j=4)[:, :, jj],
            in_=tp[:].rearrange("p (a c) -> p a c", c=32)[:, :, 0])

    def emit_softmax(bb):
        stile = sT[bb]
        pmax = smp.tile([128, 1], F32, tag="pmax", name=f"pmax{bb}")
        nc.vector.tensor_reduce(pmax[:], stile[:], axis=AX.X, op=ALU.max)
        gmax = smp.tile([128, 1], F32, tag="gmax", name=f"gmax{bb}")
        nc.gpsimd.partition_all_reduce(
            gmax[:], pmax[:], channels=128, reduce_op=bass_isa.ReduceOp.max)
        negmax = smp.tile([128, 1], F32, tag="negmax", name=f"negmax{bb}")
        nc.vector.tensor_scalar(negmax[:], gmax[:], -1.0, None, op0=ALU.mult)
        acc = smp.tile([128, 1], F32, tag="acc", name=f"acc{bb}")
        sl = s_sb[:, bb * 16 : (bb + 1) * 16]
        nc.scalar.activation(sl, stile[:], AF.Exp, bias=negmax[:],
                             accum_out=acc[:])
        gsum = smp.tile([128, 1], F32, tag="gsum", name=f"gsum{bb}")
        nc.gpsimd.partition_all_reduce(
            gsum[:], acc[:], channels=128, reduce_op=bass_isa.ReduceOp.add)
        rc = smp.tile([128, 1], F32, tag="rc", name=f"rc{bb}")
        nc.vector.reciprocal(rc[:], gsum[:])
        nc.vector.tensor_scalar(sl, sl, rc[:], 1.0, op0=ALU.mult, op1=ALU.add)

    def emit_scale_store(n, mul_eng, q):
        stage = stp.tile([128, H], F32, tag="st", name=f"st{n}")
        if mul_eng is nc.scalar:
            nc.scalar.mul(stage[:], tokh_sb[n][:], s_sb[:, n : n + 1])
        else:
            mul_eng.tensor_scalar(stage[:], tokh_sb[n][:], s_sb[:, n : n + 1],
                                  None, op0=ALU.mult)
        q.dma_start(out_t[n], stage[:])
        del tokh_sb[n]

    # per-block hook: everything that interleaves into block t's MM stream
    # after m-group m has been emitted.
    # Blocks 3 and 7 run their own v-dots in-block (2-group lag) so each
    # batch's softmax chain starts one block earlier; other blocks defer
    # v-dots into the next block's stream.
    IN_BLOCK_VD = (3, NB - 1)
    b0_load_n = [0]
    b1_load_n = [16]
    b0_store_n = [0]

    def after_group(t, m):
        # deferred v-dots of the previous block (fp8 DR, 4 per block).
        # In blocks 3/7 they are packed at m=1..4 because those blocks'
        # own v-dots follow at m=5..7 and a start=True v-dot clears the
        # whole PSUM bank's has_written bits.
        if t >= 1 and (t - 1) not in IN_BLOCK_VD:
            if t in IN_BLOCK_VD and 1 <= m <= 4:
                emit_vdot(t - 1, m - 1)
            elif t not in IN_BLOCK_VD and m in (1, 3, 5, 7):
                emit_vdot(t - 1, m // 2)
        # in-block v-dots with a 2-group lag (blocks 3 and 7)
        if t in IN_BLOCK_VD and m >= 5:
            emit_vdot(t, m - 5)
        if t == 4 and m == 0:
            emit_vdot(3, 3)
        # tokh prefetch: batch-0 tiles during blocks 0-2, batch-1 in 3-5
        if t <= 2 and m in (2, 4, 6) and b0_load_n[0] < 16:
            load_tokh(b0_load_n[0], nc.sync)
            load_tokh(b0_load_n[0] + 1, nc.gpsimd)
            b0_load_n[0] += 2
        if 3 <= t <= 5 and m in (2, 4, 6) and b1_load_n[0] < 32:
            load_tokh(b1_load_n[0], nc.sync)
            load_tokh(b1_load_n[0] + 1, nc.scalar)
            b1_load_n[0] += 2
        # batch-0 score transposes + softmax during block 4
        if t == 4 and 4 <= m <= 7:
            emit_score_tp(0, m - 4)
            if m == 7:
                emit_softmax(0)
        # batch-0 scale+stores: spread over blocks 5-7
        if ((t == 5 and m in (1, 3, 5, 7))
                or (t == 6 and m in (1, 2, 3, 5, 6, 7))
                or (t == 7 and m in (0, 1, 2, 3, 4, 5))) and b0_store_n[0] < 16:
            i = b0_store_n[0]
            emit_scale_store(i, nc.vector,
                             nc.gpsimd if i % 2 == 0 else nc.sync)
            b0_store_n[0] += 1

    # ---- main loop ----------------------------------------------------
    for t in range(NB):
        bb = t // 4
        if t == 0:
            # kp-outer over m0-3: MMs start as w8 chunks + tokT block 0
            # arrive instead of waiting for the full weight load
            mms0 = [mm_ps.tile([128, 512], F32, tag="mm", name=f"mm0_{m}")
                    for m in range(4)]
            for kp in range(KP):
                for m in range(4):
                    nc.tensor.matmul(
                        mms0[m][:],
                        w8_sb[:, kp, :, m * 128 : (m + 1) * 128],
                        tokT_sb[:, t, kp],
                        start=(kp == 0), stop=(kp == KP - 1), perf_mode=DR)
            for m in range(4):
                q, ko = divmod(m, 2)
                if ko == 0:
                    e8[(t, q)] = ep.tile([128, 2, 512], F8, tag="e", name=f"e{t}_{q}")
                nc.scalar.activation(e8[(t, q)][:, ko, :], mms0[m][:],
                                     AF.Tanh, bias=biasT[:, m, bb : bb + 1],
                                     scale=1.0 / WSCALE)
                after_group(t, m)
            mrange = range(4, MC)
        else:
            mrange = range(MC)

        for m in mrange:
            mm = mm_ps.tile([128, 512], F32, tag="mm", name=f"mm{t}_{m}")
            for kp in range(KP):
                nc.tensor.matmul(
                    mm[:],
                    w8_sb[:, kp, :, m * 128 : (m + 1) * 128],
                    tokT_sb[:, t, kp],
                    start=(kp == 0), stop=(kp == KP - 1), perf_mode=DR)
            q, ko = divmod(m, 2)
            if ko == 0:
                e8[(t, q)] = ep.tile([128, 2, 512], F8, tag="e", name=f"e{t}_{q}")
            nc.scalar.activation(e8[(t, q)][:, ko, :], mm[:], AF.Tanh,
                                 bias=biasT[:, m, bb : bb + 1],
                                 scale=1.0 / WSCALE)
            after_group(t, m)

    # ---- tail: last block's remaining v-dot, batch-1 softmax + stores -
    t = NB - 1
    emit_vdot(t, 3)
    for jj in range(4):
        emit_score_tp(1, jj)
    emit_softmax(1)
    tail_q = [nc.sync, nc.gpsimd, nc.scalar]
    tail_m = [nc.vector, nc.scalar, nc.vector, nc.gpsimd]
    for i, n in enumerate(range(16, 32)):
        emit_scale_store(n, tail_m[i % 4], tail_q[i % 3])


_CACHE = {}


def _build():
    if "nc" in _CACHE:
        return _CACHE["nc"]
    nc = bacc.Bacc("TRN2", target_bir_lowering=False, debug=False,
                   num_devices=NCORES)
    tokT_ = nc.dram_tensor("tokT", [NB, 128, KP, 2, 512], F8,
                           kind="ExternalInput").ap()
    w8_ = nc.dram_tensor("w8", [128, KP, 2, H], F8, kind="ExternalInput").ap()
    tokh_ = nc.dram_tensor("tokh", [NT, 128, H], F16,
                           kind="ExternalInput").ap()
    bT = nc.dram_tensor("biasT", [H, BPC], F32, kind="ExternalInput").ap()
    v8_ = nc.dram_tensor("v8", [128, MC, 16], F8, kind="ExternalInput").ap()
    outp = nc.dram_tensor("out", [T, H], F32, kind="ExternalOutput").ap()

    with tile.TileContext(nc) as tc:
        with ExitStack() as ctx:
            _emit(ctx, tc, outp, tokT_, w8_, tokh_, bT, v8_)
    nc.compile()
    _CACHE["nc"] = nc
    return nc


def host_bias(aspect_embedding, W, b):
    """bias[b, h] = mean_A(asp)[b] @ W2 + b  (64KB constant, on host)."""
    am = aspect_embedding.astype(np.float64).mean(axis=1)      # [B, H]
    return (am @ W.astype(np.float64)[H:] + b.astype(np.float64)).astype(
        np.float32)                                            # [B, H]


def host_pack_x(x):
    """Per-core host-side layouts of the token shard x [T, H] f32."""
    tokT = np.ascontiguousarray(
        x.reshape(NB, 512, KP, 2, 128).transpose(0, 4, 2, 3, 1)).astype(F8NP)
    tokh = x.reshape(NT, 128, H).astype(np.float16)
    return tokT, tokh


def make_in_maps(token_embeddings, aspect_embedding, W, b, v):
    bias = host_bias(aspect_embedding, W, b)
    w8 = np.ascontiguousarray(
        (W[:H].astype(np.float32) * WSCALE)
        .reshape(KP, 2, 128, H).transpose(2, 0, 1, 3)).astype(F8NP)
    v8 = np.zeros((128, MC, 16), dtype=F8NP)
    v8[:, :, 0] = v.reshape(MC, 128).T.astype(F8NP)
    in_maps = []
    for c in range(NCORES):
        x = np.ascontiguousarray(
            token_embeddings[BPC * c : BPC * (c + 1)].reshape(T, H))
        tokT, tokh = host_pack_x(x)
        in_maps.append({
            "tokT": tokT, "tokh": tokh, "w8": w8, "v8": v8,
            "biasT": np.ascontiguousarray(bias[BPC * c : BPC * (c + 1)].T),
        })
    return in_maps


def kernel(token_embeddings, aspect_embedding, W, b, v):
    token_embeddings = np.asarray(token_embeddings, dtype=np.float32)
    aspect_embedding = np.asarray(aspect_embedding, dtype=np.float32)
    W = np.asarray(W, dtype=np.float32)
    b = np.asarray(b, dtype=np.float32)
    v = np.asarray(v, dtype=np.float32)

    nc = _build()
    in_maps = make_in_maps(token_embeddings, aspect_embedding, W, b, v)
    res = run_bass_kernel_spmd(nc, in_maps, core_ids=list(range(NCORES)))
    return np.concatenate(
        [res.results[c]["out"].reshape(BPC, S, H) for c in range(NCORES)], axis=0)


# revision 17
# speedup vs baseline: 1.6621x; 1.6621x over previous
"""AspectAttention Trainium2 kernel (8 NeuronCores, batch-parallel, fp8).

out = tok * (1 + softmax_S(tanh(cat(tok, mean_A(asp)) @ W + b) @ v))

Sharding: data-parallel over batch B=16 -> 2 batches per core. Softmax is
per-(batch) row over S, so no cross-core communication is needed.

Per-core math (concat split): E^T = tanh(W1^T @ X^T + biasT), where
biasT = (mean_A(asp) @ W2 + b) is per-batch and precomputed on the host
(a 64KB constant, 0.03% of the FLOPs). scores = v^T @ E^T;
weights = softmax(scores); out = X * (1 + weights).

Layout strategy (v2): all transposes and dtype casts are done on the host.
The device receives
  - tokT  [NB, 128, KP, 2, 512] fp8: X^T pre-transposed and pre-cast, laid
    out as DoubleRow k-pair rhs slabs (one contiguous 512KB chunk per
    512-token block),
  - w8    [128, KP, 2, H] fp8: 64*W1 as DoubleRow lhsT chunks (1MB),
  - tokh  [NT, 128, H] fp16: X natural-layout for the output scaling
    (fp16 is lossless enough: ~5e-4 elementwise vs the 2e-2 gate),
  - biasT [H, BPC] f32, v8 [128, MC, 16] fp8.
This removes all PE transpose traffic and all DVE cast traffic from the
device and cuts HBM reads from 21MB to 13.6MB per core. The PE stream is
then almost pure DoubleRow matmuls at ~216ns per [256 x 128 x 512] MM.

Schedule notes:
- v-dots run as fp8 DoubleRow MMs over pair-interleaved e tiles (tanh
  writes fp8 directly into the two Ko slabs), deferred one block so the
  PSUM->tanh->vdot chain hides inside the next block's MM stream.
- scores are staged as [4, 512] rows per batch and transposed in four
  [4,128] PE transposes at batch end (N=4 each, ~free).
- block 0 runs kp-outer over m0-3 so MMs start as soon as the first w8
  chunk + tokT block land (~3us); ~18 warmup MMs keep HAM busy before.
- batch-0 scale+stores spread over blocks 5-7; batch-1 is the tail
  (8.4MB of stores, write-BW-bound).
"""

from contextlib import ExitStack

import numpy as np
import ml_dtypes

import concourse.bass as bass
import concourse.mybir as mybir
import concourse.tile as tile
from concourse import bacc, bass_isa
from concourse.bass_utils import run_bass_kernel_spmd
from concourse.masks import make_identity

B, S, H, A = 16, 2048, 1024, 8
NCORES = 8
BPC = B // NCORES          # batches per core = 2
T = BPC * S                # tokens per core = 4096
NT = T // 128              # 32 token-128 tiles per core
NB = T // 512              # 8 token-512 blocks per core
KP = 4                     # double-row k-pairs (contraction 1024 = 4*256)
MC = H // 128              # 8 output-dim chunks
WSCALE = 64.0              # W1 fp8 pre-scale

F32 = mybir.dt.float32
F16 = mybir.dt.float16
F8 = mybir.dt.float8e4
ALU = mybir.AluOpType
AF = mybir.ActivationFunctionType
AX = mybir.AxisListType
DR = mybir.MatmulPerfMode.DoubleRow

F8NP = ml_dtypes.float8_e4m3


def _emit(ctx: ExitStack, tc: "tile.TileContext", out, tokT, w8, tokh,
          biasT_in, v8_in):
    nc = tc.nc

    const = ctx.enter_context(tc.tile_pool(name="const", bufs=1))
    thp = ctx.enter_context(tc.tile_pool(name="thp", bufs=28))
    ep = ctx.enter_context(tc.tile_pool(name="ep", bufs=10))
    stp = ctx.enter_context(tc.tile_pool(name="stp", bufs=6))
    smp = ctx.enter_context(tc.tile_pool(name="smp", bufs=1))

    # PSUM: 8 banks, bank-granular: mm 4 + vd 2 + tp 2
    mm_ps = ctx.enter_context(tc.tile_pool(name="mm_ps", bufs=4, space="PSUM"))
    vd_ps = ctx.enter_context(tc.tile_pool(name="vd_ps", bufs=2, space="PSUM"))
    tp_ps = ctx.enter_context(tc.tile_pool(name="tp_ps", bufs=2, space="PSUM"))

    # ---- HAM warmup: dummy matmuls keep the PE busy from t=0 while the
    # first DMAs land. The result is copied into a scratch tile no one
    # reads for real.
    warm = const.tile([128, 512], F8)
    nc.vector.memset(warm[:], 0.0)
    scratch = const.tile([128, 1], F32)
    wps = None
    for r in range(42):
        wps = mm_ps.tile([128, 256], F32, tag="mm", name=f"warm{r % 4}")
        nc.tensor.matmul(wps[:], warm[:, 0:128], warm[:, 0:256],
                         start=True, stop=True, skip_group_check=True)
    nc.vector.tensor_copy(out=scratch[:], in_=wps[:, 0:1])

    # ---- constants / small inputs ------------------------------------
    ident = const.tile([128, 128], F32)
    make_identity(nc, ident[:])

    biasT = const.tile([128, MC, BPC], F32)      # bias[b, m*128+p] at [p,m,b]
    nc.scalar.dma_start(biasT[:], biasT_in.rearrange("(m p) b -> p m b", p=128))
    v8 = const.tile([128, MC, 16], F8)           # v[m*128+p] at [p, m, 0]
    nc.scalar.dma_start(v8[:], v8_in)

    # ---- bulk input tiles --------------------------------------------
    w8_sb = const.tile([128, KP, 2, H], F8)      # 64*W1 DR lhsT chunks
    tokT_sb = const.tile([128, NB, KP, 2, 512], F8)  # X^T DR rhs slabs

    nc.sync.dma_start(tokT_sb[:, 0], tokT[0])
    nc.gpsimd.dma_start(w8_sb[:, 0], w8[:, 0])
    nc.sync.dma_start(w8_sb[:, 1], w8[:, 1])
    nc.gpsimd.dma_start(w8_sb[:, 2], w8[:, 2])
    nc.sync.dma_start(tokT_sb[:, 1], tokT[1])
    nc.gpsimd.dma_start(w8_sb[:, 3], w8[:, 3])
    for t in range(2, NB):
        (nc.sync if t % 2 == 0 else nc.gpsimd).dma_start(
            tokT_sb[:, t], tokT[t])

    out_t = out.rearrange("(n p) h -> n p h", p=128)

    s_sb = const.tile([128, NT], F32)   # per-token-tile (1 + weight) scales
    tokh_sb = {}                        # n -> [128, H] f16 tile
    e8 = {}                             # (t, q) -> [128, 2, 512] f8 tile
    vd_row = {}                         # t -> [1, 512] psum score row
    sc4 = {}                            # bb -> [128, 512] score rows (SBUF)
    sT = {}                             # bb -> [128, 16] transposed scores
    for bb in range(BPC):
        sc4[bb] = smp.tile([128, 512], F32, tag=f"sc4_{bb}", name=f"sc4_{bb}")
        nc.vector.memset(sc4[bb][:], 0.0)
        sT[bb] = smp.tile([128, 16], F32, tag=f"sT{bb}", name=f"sT{bb}")

    def load_tokh(n, eng):
        t_ = thp.tile([128, H], F16, tag="th", name=f"th{n}")
        tokh_sb[n] = t_
        eng.dma_start(t_[:], tokh[n])

    def emit_vdot(t, q):
        if q == 0:
            vd_row[t] = vd_ps.tile([1, 512], F32, tag="vd", name=f"vd{t}")
        nc.tensor.matmul(
            vd_row[t][:], v8[:, 2 * q : 2 * q + 2, 0:1], e8[(t, q)][:],
            start=(q == 0), stop=(q == 3), perf_mode=DR,
            skip_group_check=True)
        if q == 3:
            # copy the finished row to partition 32*(t%4) of the batch's
            # score tile (engine writes must start 32-aligned)
            bb, r = divmod(t, 4)
            nc.scalar.copy(sc4[bb][32 * r : 32 * r + 1, :], vd_row[t][:])

    def emit_score_tp(bb, jj):
        tp = tp_ps.tile([128, 128], F32, tag="tp", name=f"tp{bb}_{jj}")
        nc.tensor.transpose(
            tp[:], sc4[bb][:, jj * 128 : (jj + 1) * 128], ident[:])
        nc.vector.tensor_copy(
            out=sT[bb][:].rearrange("p (b j) -> p b j", j=4)[:, :, jj],
            in_=tp[:].rearrange("p (a c) -> p a c", c=32)[:, :, 0])

    def emit_softmax(bb):
        stile = sT[bb]
        pmax = smp.tile([128, 1], F32, tag="pmax", name=f"pmax{bb}")
        nc.vector.tensor_reduce(pmax[:], stile[:], axis=AX.X, op=ALU.max)
        gmax = smp.tile([128, 1], F32, tag="gmax", name=f"gmax{bb}")
        nc.gpsimd.partition_all_reduce(
            gmax[:], pmax[:], channels=128, reduce_op=bass_isa.ReduceOp.max)
        negmax = smp.tile([128, 1], F32, tag="negmax", name=f"negmax{bb}")
        nc.vector.tensor_scalar(negmax[:], gmax[:], -1.0, None, op0=ALU.mult)
        acc = smp.tile([128, 1], F32, tag="acc", name=f"acc{bb}")
        sl = s_sb[:, bb * 16 : (bb + 1) * 16]
        nc.scalar.activation(sl, stile[:], AF.Exp, bias=negmax[:],
                             accum_out=acc[:])
        gsum = smp.tile([128, 1], F32, tag="gsum", name=f"gsum{bb}")
        nc.gpsimd.partition_all_reduce(
            gsum[:], acc[:], channels=128, reduce_op=bass_isa.ReduceOp.add)
        rc = smp.tile([128, 1], F32, tag="rc", name=f"rc{bb}")
        nc.vector.reciprocal(rc[:], gsum[:])
        nc.vector.tensor_scalar(sl, sl, rc[:], 1.0, op0=ALU.mult, op1=ALU.add)

    def emit_scale_store(n, mul_eng, q):
        stage = stp.tile([128, H], F32, tag="st", name=f"st{n}")
        if mul_eng is nc.scalar:
            nc.scalar.mul(stage[:], tokh_sb[n][:], s_sb[:, n : n + 1])
        else:
            mul_eng.tensor_scalar(stage[:], tokh_sb[n][:], s_sb[:, n : n + 1],
                                  None, op0=ALU.mult)
        q.dma_start(out_t[n], stage[:])
        del tokh_sb[n]

    # per-block hook: everything that interleaves into block t's MM stream
    # after m-group m has been emitted.
    # Blocks 3 and 7 run their own v-dots in-block (2-group lag) so each
    # batch's softmax chain starts one block earlier; other blocks defer
    # v-dots into the next block's stream.
    IN_BLOCK_VD = (3, NB - 1)
    b0_load_n = [0]
    b1_load_n = [16]
    b0_store_n = [0]

    def after_group(t, m):
        # deferred v-dots of the previous block (fp8 DR, 4 per block).
        # In blocks 3/7 they are packed at m=1..4 because those blocks'
        # own v-dots follow at m=5..7 and a start=True v-dot clears the
        # whole PSUM bank's has_written bits.
        if t >= 1 and (t - 1) not in IN_BLOCK_VD:
            if t in IN_BLOCK_VD and 1 <= m <= 4:
                emit_vdot(t - 1, m - 1)
            elif t not in IN_BLOCK_VD and m in (1, 3, 5, 7):
                emit_vdot(t - 1, m // 2)
        # in-block v-dots with a 2-group lag (blocks 3 and 7)
        if t in IN_BLOCK_VD and m >= 5:
            emit_vdot(t, m - 5)
        if t == 4 and m == 0:
            emit_vdot(3, 3)
        # tokh prefetch: batch-0 tiles during blocks 0-2, batch-1 in 3-5
        if t <= 2 and m in (2, 4, 6) and b0_load_n[0] < 16:
            load_tokh(b0_load_n[0], nc.sync)
            load_tokh(b0_load_n[0] + 1, nc.gpsimd)
            b0_load_n[0] += 2
        if 3 <= t <= 5 and m in (2, 4, 6) and b1_load_n[0] < 32:
            load_tokh(b1_load_n[0], nc.sync)
            load_tokh(b1_load_n[0] + 1, nc.scalar)
            b1_load_n[0] += 2
        # batch-0 score transposes + softmax during block 4
        if t == 4 and 4 <= m <= 7:
            emit_score_tp(0, m - 4)
            if m == 7:
                emit_softmax(0)
        # batch-0 scale+stores: spread over blocks 5-7
        if ((t == 5 and m in (1, 3, 5, 7))
                or (t == 6 and m in (1, 2, 3, 5, 6, 7))
                or (t == 7 and m in (0, 1, 2, 3, 4, 5))) and b0_store_n[0] < 16:
            i = b0_store_n[0]
            emit_scale_store(i, nc.vector,
                             nc.gpsimd if i % 2 == 0 else nc.sync)
            b0_store_n[0] += 1

    # ---- main loop ----------------------------------------------------
    for t in range(NB):
        bb = t // 4
        if t == 0:
            # kp-outer over m0-3: MMs start as w8 chunks + tokT block 0
            # arrive instead of waiting for the full weight load
            mms0 = [mm_ps.tile([128, 512], F32, tag="mm", name=f"mm0_{m}")
                    for m in range(4)]
            for kp in range(KP):
                for m in range(4):
                    nc.tensor.matmul(
                        mms0[m][:],
                        w8_sb[:, kp, :, m * 128 : (m + 1) * 128],
                        tokT_sb[:, t, kp],
                        start=(kp == 0), stop=(kp == KP - 1), perf_mode=DR)
            for m in range(4):
                q, ko = divmod(m, 2)
                if ko == 0:
                    e8[(t, q)] = ep.tile([128, 2, 512], F8, tag="e", name=f"e{t}_{q}")
                nc.scalar.activation(e8[(t, q)][:, ko, :], mms0[m][:],
                                     AF.Tanh, bias=biasT[:, m, bb : bb + 1],
                                     scale=1.0 / WSCALE)
                after_group(t, m)
            mrange = range(4, MC)
        else:
            mrange = range(MC)

        for m in mrange:
            mm = mm_ps.tile([128, 512], F32, tag="mm", name=f"mm{t}_{m}")
            for kp in range(KP):
                nc.tensor.matmul(
                    mm[:],
                    w8_sb[:, kp, :, m * 128 : (m + 1) * 128],
                    tokT_sb[:, t, kp],
                    start=(kp == 0), stop=(kp == KP - 1), perf_mode=DR)
            q, ko = divmod(m, 2)
            if ko == 0:
                e8[(t, q)] = ep.tile([128, 2, 512], F8, tag="e", name=f"e{t}_{q}")
            nc.scalar.activation(e8[(t, q)][:, ko, :], mm[:], AF.Tanh,
                                 bias=biasT[:, m, bb : bb + 1],
                                 scale=1.0 / WSCALE)
            after_group(t, m)

    # ---- tail: last block's remaining v-dot, batch-1 softmax + stores -
    t = NB - 1
    emit_vdot(t, 3)
    for jj in range(4):
        emit_score_tp(1, jj)
    emit_softmax(1)
    # gpsimd tensor_scalar is a ~18us software loop per tile: never use it
    # for the scale muls (DVE ~0.9us, ACT ~1.5us)
    tail_q = [nc.sync, nc.gpsimd, nc.scalar]
    tail_m = [nc.vector, nc.vector, nc.scalar]
    for i, n in enumerate(range(16, 32)):
        emit_scale_store(n, tail_m[i % 3], tail_q[i % 3])


_CACHE = {}


def _build():
    if "nc" in _CACHE:
        return _CACHE["nc"]
    nc = bacc.Bacc("TRN2", target_bir_lowering=False, debug=False,
                   num_devices=NCORES)
    tokT_ = nc.dram_tensor("tokT", [NB, 128, KP, 2, 512], F8,
                           kind="ExternalInput").ap()
    w8_ = nc.dram_tensor("w8", [128, KP, 2, H], F8, kind="ExternalInput").ap()
    tokh_ = nc.dram_tensor("tokh", [NT, 128, H], F16,
                           kind="ExternalInput").ap()
    bT = nc.dram_tensor("biasT", [H, BPC], F32, kind="ExternalInput").ap()
    v8_ = nc.dram_tensor("v8", [128, MC, 16], F8, kind="ExternalInput").ap()
    outp = nc.dram_tensor("out", [T, H], F32, kind="ExternalOutput").ap()

    with tile.TileContext(nc) as tc:
        with ExitStack() as ctx:
            _emit(ctx, tc, outp, tokT_, w8_, tokh_, bT, v8_)
    nc.compile()
    _CACHE["nc"] = nc
    return nc


def host_bias(aspect_embedding, W, b):
    """bias[b, h] = mean_A(asp)[b] @ W2 + b  (64KB constant, on host)."""
    am = aspect_embedding.astype(np.float64).mean(axis=1)      # [B, H]
    return (am @ W.astype(np.float64)[H:] + b.astype(np.float64)).astype(
        np.float32)                                            # [B, H]


def host_pack_x(x):
    """Per-core host-side layouts of the token shard x [T, H] f32."""
    tokT = np.ascontiguousarray(
        x.reshape(NB, 512, KP, 2, 128).transpose(0, 4, 2, 3, 1)).astype(F8NP)
    tokh = x.reshape(NT, 128, H).astype(np.float16)
    return tokT, tokh


def make_in_maps(token_embeddings, aspect_embedding, W, b, v):
    bias = host_bias(aspect_embedding, W, b)
    w8 = np.ascontiguousarray(
        (W[:H].astype(np.float32) * WSCALE)
        .reshape(KP, 2, 128, H).transpose(2, 0, 1, 3)).astype(F8NP)
    v8 = np.zeros((128, MC, 16), dtype=F8NP)
    v8[:, :, 0] = v.reshape(MC, 128).T.astype(F8NP)
    in_maps = []
    for c in range(NCORES):
        x = np.ascontiguousarray(
            token_embeddings[BPC * c : BPC * (c + 1)].reshape(T, H))
        tokT, tokh = host_pack_x(x)
        in_maps.append({
            "tokT": tokT, "tokh": tokh, "w8": w8, "v8": v8,
            "biasT": np.ascontiguousarray(bias[BPC * c : BPC * (c + 1)].T),
        })
    return in_maps


def kernel(token_embeddings, aspect_embedding, W, b, v):
    token_embeddings = np.asarray(token_embeddings, dtype=np.float32)
    aspect_embedding = np.asarray(aspect_embedding, dtype=np.float32)
    W = np.asarray(W, dtype=np.float32)
    b = np.asarray(b, dtype=np.float32)
    v = np.asarray(v, dtype=np.float32)

    nc = _build()
    in_maps = make_in_maps(token_embeddings, aspect_embedding, W, b, v)
    res = run_bass_kernel_spmd(nc, in_maps, core_ids=list(range(NCORES)))
    return np.concatenate(
        [res.results[c]["out"].reshape(BPC, S, H) for c in range(NCORES)], axis=0)
